# revision 14
# baseline (speedup 1.0000x reference)
"""Mean-Average-Precision (detection mAP) Bass kernel for 8 TRN2 NeuronCores.

Sharding: class dimension across the 8 cores (10 classes each, SPMD one NEFF).
Per class the kernel scans the [P=30080 x G=800] pair space in 235 tiles of
[128 preds x 800 gts]:
  - exact hot test  3*inter > A+B  (equivalent to IoU > 0.5)
  - argmax over gts via ln(inter)-ln(A+B)  (monotone in IoU)
  - per-gt first-claim = max over claiming preds of (score+10)  (claims only at
    the pred's argmax gt, gated on the pred being hot anywhere)
Then per-gt winner scores are ranked against all scores (count of strictly
greater scores = sorted position) and against each other (TP rank), and the
trapezoidal AP is accumulated from the <=800 TP terms per class.
"""

import os
import sys

import numpy as np

sys.path.insert(0, "/opt/trn_rl_repo")

C, P, G = 80, 30000, 800
NCORES = 8
CPC = C // NCORES          # classes per core
NT = 235                   # pred tiles per class
PPAD = 128 * NT            # 30080
GP = 896                   # padded gt count (7 * 128)
SCPAD = 32768              # padded score count for rank compare (64 * 512)
RCH = 4096                 # rank-compare chunk (free dim)
BIGNEG = -3.4e38

if os.environ.get("MAP_KERNEL_SMALL"):   # fast API smoke test config
    NT = int(os.environ.get("MAP_KERNEL_NT", "4"))
    CPC = int(os.environ.get("MAP_KERNEL_CPC", "1"))
    PPAD = 128 * NT
    SCPAD = 4096
    RCH = 2048


def _build_core_inputs(pred_boxes: np.ndarray, gt_boxes: np.ndarray):
    """Host-side layout/sharding only: slice classes per core, pad, transpose."""
    ones1 = np.ones((1, 128), np.float32)
    ones128 = np.ones((128, 1), np.float32)
    ident = np.eye(128, dtype=np.float32)
    in_maps = []
    for core in range(NCORES):
        cls = slice(core * CPC, (core + 1) * CPC)
        pr = np.asarray(pred_boxes[cls], np.float32)         # [CPC, P, 7]
        gt = np.asarray(gt_boxes[cls], np.float32)           # [CPC, G, 7]
        pad = np.zeros((CPC, PPAD - P, 7), np.float32)
        pad[:, :, 2] = -1.0                                  # score
        pad[:, :, 3] = -4096.0; pad[:, :, 4] = -4096.0       # x1 y1
        pad[:, :, 5] = -4000.0; pad[:, :, 6] = -4000.0       # x2 y2
        prp = np.concatenate([pr, pad], axis=1)              # [CPC, PPAD, 7]
        r = prp.reshape(CPC, 128, NT, 7)
        preds = np.empty((CPC, 128, 6, NT), np.float32)
        preds[:, :, 0] = r[..., 2]                           # score
        preds[:, :, 1] = r[..., 3]                           # x1
        preds[:, :, 2] = r[..., 4]                           # y1
        preds[:, :, 3] = r[..., 5]                           # x2
        preds[:, :, 4] = r[..., 6]                           # y2
        preds[:, :, 5] = (r[..., 5] - r[..., 3]) * (r[..., 6] - r[..., 4])  # A
        gts = np.empty((CPC, 5, G), np.float32)
        gts[:, 0] = gt[..., 3]; gts[:, 1] = gt[..., 4]
        gts[:, 2] = gt[..., 5]; gts[:, 3] = gt[..., 6]
        gts[:, 4] = (gt[..., 5] - gt[..., 3]) * (gt[..., 6] - gt[..., 4])   # B
        scflat = np.full((CPC, SCPAD), -1.0, np.float32)
        scflat[:, :PPAD] = prp[:, :, 2]
        in_maps.append({
            "preds": np.ascontiguousarray(preds),
            "gts": np.ascontiguousarray(gts),
            "scflat": np.ascontiguousarray(scflat),
            "ones1": ones1,
            "ones128": ones128,
            "ident": ident,
        })
    return in_maps


def _build_kernel():
    import concourse.bass as bass
    import concourse.mybir as mybir
    from concourse import bacc, tile
    from concourse.bass_utils import axon_active

    dt = mybir.dt.float32
    Alu = mybir.AluOpType
    Act = mybir.ActivationFunctionType

    nc = bacc.Bacc(
        "TRN2",
        target_bir_lowering=False,
        debug=False,
        num_devices=NCORES,
    )
    preds_d = nc.dram_tensor("preds", [CPC, 128, 6, NT], dt, kind="ExternalInput").ap()
    gts_d = nc.dram_tensor("gts", [CPC, 5, G], dt, kind="ExternalInput").ap()
    scflat_d = nc.dram_tensor("scflat", [CPC, SCPAD], dt, kind="ExternalInput").ap()
    ones1_d = nc.dram_tensor("ones1", [1, 128], dt, kind="ExternalInput").ap()
    ones128_d = nc.dram_tensor("ones128", [128, 1], dt, kind="ExternalInput").ap()
    ident_d = nc.dram_tensor("ident", [128, 128], dt, kind="ExternalInput").ap()
    out_d = nc.dram_tensor("out", [1, 1], dt, kind="ExternalOutput").ap()

    with tile.TileContext(nc) as tc:
        with (
            tc.tile_pool(name="consts", bufs=1) as consts,
            tc.tile_pool(name="preds", bufs=2) as predp,
            tc.tile_pool(name="gtbc", bufs=1) as gtbcp,
            tc.tile_pool(name="acc", bufs=2) as accp,
            tc.tile_pool(name="work", bufs=2) as work,
            tc.tile_pool(name="cols", bufs=4) as cols,
            tc.tile_pool(name="strips", bufs=2) as strips,
            tc.tile_pool(name="rank", bufs=1) as rankp,
            tc.tile_pool(name="psum", bufs=2, space="PSUM") as psum,
            tc.tile_pool(name="dram", bufs=2, space="DRAM") as dramp,
        ):
            ones1 = consts.tile([1, 128], dt)
            ones128 = consts.tile([128, 1], dt)
            ident = consts.tile([128, 128], dt)
            nc.sync.dma_start(ones1[:], ones1_d[:])
            nc.sync.dma_start(ones128[:], ones128_d[:])
            nc.sync.dma_start(ident[:], ident_d[:])

            apacc = consts.tile([128, 7], dt, tag="apacc")
            nc.vector.memset(apacc[:], 0.0)
            zbias = consts.tile([128, 1], dt, tag="zbias")
            nc.vector.memset(zbias[:], 0.0)
            eps30 = consts.tile([128, 1], dt, tag="eps30")
            nc.vector.memset(eps30[:], 1e-30)

            for cls in range(CPC):
                # ---- load per-class pred fields and gt rows ----
                predt = predp.tile([128, 6, NT], dt, tag="predt")
                nc.sync.dma_start(predt[:], preds_d[cls])
                # ---- broadcast gt rows to 128 partitions via PE ----
                gbc = []   # x1 y1 x2 y2 B as [128, G]
                for f in range(5):
                    gtrow = predp.tile([1, G], dt, tag="gtrow")
                    nc.sync.dma_start(gtrow[:], gts_d[cls, f : f + 1, :])
                    t = gtbcp.tile([128, G], dt, tag=f"gbc{f}")
                    for j, (lo, hi) in enumerate(((0, 512), (512, 800))):
                        pt = psum.tile([128, hi - lo], dt, tag="pbc")
                        nc.tensor.matmul(pt[:], ones1[:], gtrow[0:1, lo:hi],
                                         start=True, stop=True)
                        if f % 2 == 0:
                            nc.scalar.activation(t[:, lo:hi], pt[:], Act.Relu, bias=zbias[:])
                        else:
                            nc.vector.tensor_copy(t[:, lo:hi], pt[:])
                    gbc.append(t)
                xg1b, yg1b, xg2b, yg2b, bgb = gbc

                accgt = accp.tile([128, GP], dt, tag="accgt")
                nc.vector.memset(accgt[:], 0.0)
                sc10 = predp.tile([128, NT], dt, tag="sc10")
                nc.vector.tensor_scalar(sc10[:], predt[:, 0, :], 10.0, None, Alu.add)

                # ---- pair scan: 235 tiles of [128 preds x 800 gts] ----
                for n in range(NT):
                    sc = predt[:, 0, n : n + 1]
                    x1p = predt[:, 1, n : n + 1]
                    y1p = predt[:, 2, n : n + 1]
                    x2p = predt[:, 3, n : n + 1]
                    y2p = predt[:, 4, n : n + 1]
                    ap_ = predt[:, 5, n : n + 1]

                    u = work.tile([128, G], dt, tag="u")
                    nc.gpsimd.tensor_scalar(u[:], xg2b[:], x2p, None, Alu.min)
                    nwx = work.tile([128, G], dt, tag="nwx")
                    nc.vector.scalar_tensor_tensor(
                        nwx[:], xg1b[:], x1p, u[:], Alu.max, Alu.subtract)
                    rwx3 = work.tile([128, G], dt, tag="rwx3")
                    nc.scalar.activation(rwx3[:], nwx[:], Act.Relu, bias=zbias[:], scale=-3.0)

                    sm = work.tile([128, G], dt, tag="sm")
                    nc.gpsimd.tensor_scalar(sm[:], yg2b[:], y2p, None, Alu.min)
                    nwy = work.tile([128, G], dt, tag="nwy")
                    nc.vector.scalar_tensor_tensor(
                        nwy[:], yg1b[:], y1p, sm[:], Alu.max, Alu.subtract)
                    rwy = work.tile([128, G], dt, tag="rwy")
                    nc.scalar.activation(rwy[:], nwy[:], Act.Relu, bias=zbias[:], scale=-1.0)

                    inter3 = work.tile([128, G], dt, tag="inter3")
                    nc.vector.tensor_tensor(inter3[:], rwx3[:], rwy[:], Alu.mult)

                    lni = work.tile([128, G], dt, tag="lni")
                    nc.scalar.activation(lni[:], inter3[:], Act.Ln, bias=eps30[:])
                    lnab = work.tile([128, G], dt, tag="lnab")
                    nc.scalar.activation(lnab[:], bgb[:], Act.Ln, bias=ap_)
                    keydiff = work.tile([128, G], dt, tag="keydiff")
                    bestkey = cols.tile([128, 1], dt, tag="bestkey")
                    nc.vector.tensor_tensor(keydiff[:], lni[:], lnab[:], Alu.subtract)
                    nc.vector.tensor_reduce(bestkey[:], keydiff[:], mybir.AxisListType.X, Alu.max)

                    mscore = cols.tile([128, 1], dt, tag="mscore")
                    nc.vector.scalar_tensor_tensor(
                        mscore[:], bestkey[:], 0.0, sc10[:, n : n + 1],
                        Alu.is_gt, Alu.mult)

                    claims = work.tile([128, G], dt, tag="claims")
                    nc.vector.tensor_scalar(
                        claims[:], keydiff[:], bestkey[:], mscore[:],
                        Alu.is_ge, Alu.mult)
                    nc.vector.tensor_tensor(
                        accgt[:, 0:G], accgt[:, 0:G], claims[:], Alu.max)

                # ---- per-gt winner score: transpose + free-dim max ----
                wsraw = strips.tile([128, 7], dt, tag="wsraw")
                for cch in range(7):
                    pt = psum.tile([128, 128], dt, tag="ptr")
                    nc.tensor.transpose(pt[:], accgt[:, cch * 128 : (cch + 1) * 128],
                                        ident[:])
                    nc.vector.tensor_reduce(
                        wsraw[:, cch : cch + 1], pt[:], mybir.AxisListType.X, Alu.max)
                wstrue = strips.tile([128, 7], dt, tag="wstrue")
                nc.vector.tensor_scalar(wstrue[:], wsraw[:], 10.0, None, Alu.subtract)

                # ---- bounce winner scores to a [1, 896] row, broadcast ----
                wsdram = dramp.tile([1, GP], dt, tag="wsdram")
                nc.sync.dma_start(
                    wsdram[:].rearrange("o (p c) -> (o p) c", p=128, c=7), wsraw[:])
                wsrow = strips.tile([1, GP], dt, tag="wsrow")
                nc.sync.dma_start(wsrow[:], wsdram[:])
                wsrep = rankp.tile([128, GP], dt, tag="wsrep")
                for j, (lo, hi) in enumerate(((0, 512), (512, GP))):
                    pt = psum.tile([128, hi - lo], dt, tag="pbc")
                    nc.tensor.matmul(pt[:], ones1[:], wsrow[0:1, lo:hi],
                                     start=True, stop=True)
                    nc.vector.tensor_copy(wsrep[:, lo:hi], pt[:])

                # ---- TP rank among winners: k-1 = #(ws' > ws_g) ----
                kcnt = strips.tile([128, 7], dt, tag="kcnt")
                scr9 = rankp.tile([128, RCH], dt, tag="scr9")
                scr9b = rankp.tile([128, RCH], dt, tag="scr9b")
                for cch in range(7):
                    nc.vector.tensor_scalar(
                        scr9[:, 0:GP], wsrep[:], wsraw[:, cch : cch + 1], None,
                        Alu.is_gt)
                    nc.scalar.activation(scr9b[:, 0:GP], scr9[:, 0:GP], Act.Relu,
                                         bias=zbias[:],
                                         accum_out=kcnt[:, cch : cch + 1])

                # ---- global rank: cnt = #(score_q > ws_g) over all preds ----
                cntp = strips.tile([128, 7, 8], dt, tag="cntp")
                for sch in range(SCPAD // RCH):
                    scrow = rankp.tile([1, RCH], dt, tag="scrow")
                    nc.sync.dma_start(
                        scrow[:], scflat_d[cls, sch * RCH : (sch + 1) * RCH][None, :])
                    screp = rankp.tile([128, RCH], dt, tag="screp")
                    for j in range(RCH // 512):
                        pt = psum.tile([128, 512], dt, tag="pbc")
                        nc.tensor.matmul(pt[:], ones1[:],
                                         scrow[0:1, j * 512 : (j + 1) * 512],
                                         start=True, stop=True)
                        if j % 2 == 0:
                            nc.vector.tensor_copy(screp[:, j * 512 : (j + 1) * 512], pt[:])
                        else:
                            nc.scalar.activation(screp[:, j * 512 : (j + 1) * 512],
                                                 pt[:], Act.Relu, bias=zbias[:])
                    scscr = rankp.tile([128, RCH], dt, tag="scr9")
                    scscrb = rankp.tile([128, RCH], dt, tag="scr9b")
                    for cch in range(7):
                        nc.vector.tensor_scalar(
                            scscr[:], screp[:], wstrue[:, cch : cch + 1], None,
                            Alu.is_gt)
                        nc.scalar.activation(scscrb[:], scscr[:], Act.Relu,
                                             bias=zbias[:],
                                             accum_out=cntp[:, cch, sch : sch + 1])

                cnt = strips.tile([128, 7], dt, tag="cnt")
                for cch in range(7):
                    nc.vector.tensor_reduce(
                        cnt[:, cch : cch + 1], cntp[:, cch, :],
                        mybir.AxisListType.X, Alu.add)

                # ---- AP terms ----
                kk = strips.tile([128, 7], dt, tag="kk")
                nc.vector.tensor_scalar(kk[:], kcnt[:], 1.0, None, Alu.add)
                ip = strips.tile([128, 7], dt, tag="ip")
                nc.vector.tensor_scalar(ip[:], cnt[:], 1e-9, None, Alu.add)
                i1 = strips.tile([128, 7], dt, tag="i1")
                nc.vector.tensor_scalar(i1[:], cnt[:], 1.0 + 1e-9, None, Alu.add)
                r1 = strips.tile([128, 7], dt, tag="r1")
                nc.vector.reciprocal(r1[:], ip[:])
                r2 = strips.tile([128, 7], dt, tag="r2")
                nc.vector.reciprocal(r2[:], i1[:])
                m1 = strips.tile([128, 7], dt, tag="m1")
                nc.vector.scalar_tensor_tensor(
                    m1[:], kk[:], 1.0, r1[:], Alu.subtract, Alu.mult)
                m2 = strips.tile([128, 7], dt, tag="m2")
                nc.vector.tensor_tensor(m2[:], kk[:], r2[:], Alu.mult)
                t0 = strips.tile([128, 7], dt, tag="t0")
                nc.vector.tensor_tensor(t0[:], m1[:], m2[:], Alu.add)
                gv = strips.tile([128, 7], dt, tag="gv")
                nc.vector.tensor_scalar(gv[:], wsraw[:], 9.0, None, Alu.is_ge)
                gi = strips.tile([128, 7], dt, tag="gi")
                nc.vector.tensor_scalar(gi[:], cnt[:], 1.0, None, Alu.is_ge)
                gg = strips.tile([128, 7], dt, tag="gg")
                nc.vector.tensor_tensor(gg[:], gv[:], gi[:], Alu.mult)
                t1_ = strips.tile([128, 7], dt, tag="t1_")
                nc.vector.tensor_tensor(t1_[:], t0[:], gg[:], Alu.mult)
                t2_ = strips.tile([128, 7], dt, tag="t2_")
                nc.vector.tensor_scalar(
                    t2_[:], t1_[:], float(np.float32(0.5) / np.float32(G + 1e-9)),
                    None, Alu.mult)
                nc.vector.tensor_tensor(apacc[:], apacc[:], t2_[:], Alu.add)

            # ---- final reduction: sum apacc -> scalar ----
            aprow = consts.tile([128, 1], dt, tag="aprow")
            nc.vector.tensor_reduce(aprow[:], apacc[:], mybir.AxisListType.X, Alu.add)
            pt = psum.tile([1, 1], dt, tag="pfin")
            nc.tensor.matmul(pt[:], aprow[:], ones128[:], start=True, stop=True)
            res = consts.tile([1, 1], dt, tag="res")
            nc.vector.tensor_copy(res[:], pt[:])
            nc.sync.dma_start(out_d[:], res[:])

    nc.compile()
    return nc


def kernel(pred_boxes: np.ndarray, gt_boxes: np.ndarray) -> np.ndarray:
    from concourse.bass_utils import run_bass_kernel_spmd

    in_maps = _build_core_inputs(pred_boxes, gt_boxes)
    nc = _build_kernel()
    trace = bool(os.environ.get("MAP_KERNEL_TRACE"))
    try:
        res = run_bass_kernel_spmd(nc, in_maps, core_ids=list(range(NCORES)),
                                   trace=trace)
    except ModuleNotFoundError:
        # no NTFF profile hook in this environment -- run untraced
        res = run_bass_kernel_spmd(nc, in_maps, core_ids=list(range(NCORES)))
    global _last_exec_ns
    _last_exec_ns = res.exec_time_ns
    total = np.float32(0.0)
    for i in range(NCORES):
        total += np.float32(res.results[i]["out"].reshape(-1)[0])
    return np.asarray(total / np.float32(C), dtype=np.float32)


_last_exec_ns = None


if __name__ == "__main__":
    d = np.load("/tmp/inputs.npz")
    out = kernel(d["pb"], d["gb"])
    print("kernel out:", out)
